# revision 34
# baseline (speedup 1.0000x reference)
"""CausalTemporalAttnBlock Trainium2 kernel.

Problem: out = x + Wp @ attn(norm(x)) + bp, where norm is GroupNorm(1 group)
over (c,t,h,w) per batch, attention is causal over t, independent per (b,h,w).
Shapes: x (2, 512, 64, 32, 32) fp32; four (512,512) weights + biases.

Strategy (8 NeuronCores, zero communication except an 8-byte AllReduce for
the GroupNorm stats):
  - core i handles batch i//4, h-rows [8*(i%4), 8*(i%4)+8), all w: 256 (h,w)
    locations per core. Host re-lays the shard w-major:
    [8 h-rows][512 c][32 w * 64 t], so one attention group (8 w-locations)
    is a contiguous 512-column slice. Whole matmul datapath bf16 (fp32 PSUM).
  - Channel mixing commutes with temporal attention, so the output
    projection folds into the V projection on the host: Apv = Wp @ Av and
        out = x + r * Apv (x p^T) + dp,   dp = Wp dv + bp
    (softmax rows sum to 1, so the V-path affine passes through attention
    as a per-channel constant). This deletes a full (512,512) projection.
  - Q/K fold into one bilinear form: scores = x^T G x with G = Ak^T Aq,
    so a single Y = G x projection replaces both Q and K.
  - The surviving score affine term h[s] = x^T v0 rides as a 129th moving
    column in each score matmul (v0 appended after each pair's 128 t-cols),
    landing in PSUM next to the scores; the Exp activation consumes it as
    a per-partition bias AP. No dedicated PE work for it at all.
  - Locations are processed in PAIRS sharing the 128-wide stationary
    operand: PV^T for 2 locations as one matmul chain, scores S^T per pair
    as [128,129] chains (cross terms masked off), AV as full-K [128,128]
    matmuls (masked zeros in the normalized weights kill cross terms).
  - GroupNorm stats run entirely OFF the PE (DVE reduce for sums;
    Square/sumsq split across Scalar, GpSimd and DVE). While the stats
    pass streams x from HBM, the PE prefills the first 6 groups'
    Y/PV projections, so the collective latency hides under matmuls.
  - Softmax: no max-subtraction (scores are O(1)); exp applies the r^2
    the bilinear form is missing; normalization r/Z is multiplied into
    the masked exp weights (bf16) before the AV matmuls, so the AV
    eviction is a single fused (psum + dp) + x DVE op per chunk.
"""

import numpy as np
import ml_dtypes

import concourse.bass as bass
import concourse.tile as tile
from concourse import bacc, bass_isa, mybir
from concourse.bass_utils import run_bass_kernel_spmd

P = 128
B, C, T, H, W = 2, 512, 64, 32, 32
NCORES = 8
HSH = H // 4          # 8 h-rows per core
CCH = C // P          # 4 c chunks
GRP = 8               # locations per attention group
NGRP = W // GRP       # 4 groups per h-row block
NPR = GRP // 2        # 4 location-pairs per group
WT = W * T            # 2048 free columns per (h-row, c) plane
YW = NPR * (P + 1)    # 516: yg tile width (4 pair-blocks of 128 t-cols + v0)
RES = 4               # h-row blocks kept resident in SBUF
EPS = 1e-6

f32 = mybir.dt.float32
bf16 = mybir.dt.bfloat16
AX = mybir.AxisListType.X
ALU = mybir.AluOpType
AF = mybir.ActivationFunctionType
BF = ml_dtypes.bfloat16

# groups prefilled (Y/PV projections) during the stats pass, as (blk, g)
PREFILL = [(0, 0), (0, 1), (0, 2), (0, 3), (1, 0), (1, 1)]


def build_nc(num_cores=NCORES, nblk=HSH, norm_n=None, replica_groups=None,
             use_collective=True):
    if norm_n is None:
        norm_n = C * T * H * W
    if replica_groups is None:
        replica_groups = [[0, 1, 2, 3], [4, 5, 6, 7]]
    nc = bacc.Bacc("TRN2", target_bir_lowering=False, debug=False,
                   num_devices=num_cores)

    xs = nc.declare_dram_parameter("xs", [nblk, C, WT], bf16, isOutput=False)
    wts = {}
    for nm in ("y", "pv"):
        wts[nm] = nc.declare_dram_parameter(f"w{nm}t", [C, C], bf16,
                                            isOutput=False)
    w1col = nc.declare_dram_parameter("w1col", [P, CCH], f32, isOutput=False)
    w2col = nc.declare_dram_parameter("w2col", [P, CCH], f32, isOutput=False)
    pv1col = nc.declare_dram_parameter("pv1col", [P, CCH], f32, isOutput=False)
    pv2col = nc.declare_dram_parameter("pv2col", [P, CCH], f32, isOutput=False)
    maskp = nc.declare_dram_parameter("maskt", [P, NPR * P], bf16,
                                      isOutput=False)
    ones_mat_b = nc.declare_dram_parameter("ones_mat_b", [P, P], bf16,
                                           isOutput=False)
    ones_col_b = nc.declare_dram_parameter("ones_col_b", [P, 1], bf16,
                                           isOutput=False)
    outp = nc.declare_dram_parameter("out", [nblk, C, WT], f32, isOutput=True)
    cc_in = nc.dram_tensor("cc_in", [1, 2], f32)
    cc_out = nc.dram_tensor("cc_out", [1, 2], f32)

    with tile.TileContext(nc) as tc:
        with (
            tc.tile_pool(name="const", bufs=1) as const,
            tc.tile_pool(name="scal", bufs=1) as sc,
            tc.tile_pool(name="xres", bufs=1) as xres,
            tc.tile_pool(name="xtr", bufs=2) as xtr,
            tc.tile_pool(name="sqp", bufs=1) as sqp,
            tc.tile_pool(name="pfy", bufs=1) as pfy,
            tc.tile_pool(name="pfv", bufs=1) as pfv,
            tc.tile_pool(name="ygp", bufs=12) as ygp,
            tc.tile_pool(name="vtp", bufs=16) as vtpool,
            tc.tile_pool(name="spool", bufs=2) as spool,
            tc.tile_pool(name="rzp", bufs=2) as rzp,
            tc.tile_pool(name="opool", bufs=3) as opool,
            tc.tile_pool(name="pp", bufs=3, space="PSUM") as pp,
            tc.tile_pool(name="pss", bufs=2, space="PSUM") as pss,
            tc.tile_pool(name="scp", bufs=1, space="PSUM") as scp,
            tc.tile_pool(name="psm", bufs=1, space="PSUM") as psm,
        ):
            # ---------- constants ----------
            w_sb = {}
            for nm in ("y", "pv"):
                for ci in range(CCH):
                    t = const.tile([P, C], bf16, tag=f"w{nm}{ci}")
                    nc.sync.dma_start(t[:], wts[nm][ci * P:(ci + 1) * P, :])
                    w_sb[nm, ci] = t
            w1_sb = const.tile([P, CCH], f32, tag="w1col")
            nc.sync.dma_start(w1_sb[:], w1col[:])
            w2_sb = const.tile([P, CCH], f32, tag="w2col")
            nc.sync.dma_start(w2_sb[:], w2col[:])
            pv1_sb = const.tile([P, CCH], f32, tag="pv1col")
            nc.sync.dma_start(pv1_sb[:], pv1col[:])
            pv2_sb = const.tile([P, CCH], f32, tag="pv2col")
            nc.sync.dma_start(pv2_sb[:], pv2col[:])
            mask_sb = const.tile([P, NPR * P], bf16, tag="maskt")
            nc.sync.dma_start(mask_sb[:], maskp[:])
            omb_sb = const.tile([P, P], bf16, tag="omb")
            nc.sync.dma_start(omb_sb[:], ones_mat_b[:])
            ocb_sb = const.tile([P, 1], bf16, tag="ocb")
            nc.sync.dma_start(ocb_sb[:], ones_col_b[:])

            ssq = sc.tile([P, nblk * CCH], f32, tag="ssq")
            ssq2 = sc.tile([P, nblk * CCH], f32, tag="ssq2")

            # prefill projection outputs, keyed (blk, g)
            pf_tiles = {}

            def emit_projY(xb, cs, yg_t):
                # Y = G^T x: per co-chunk, chain over ci; two chains
                # interleaved (alternating PSUM banks) so one chain's
                # systolic drain hides under the other's streaming. Evict to
                # the 4x129-strided yg layout (col 128 of each pair block is
                # written later with v0)
                for c2 in range(CCH // 2):
                    pss2 = [pp.tile([P, 512], f32, tag="pp", name="pp")
                            for _ in range(2)]
                    for ci in range(CCH):
                        for k in range(2):
                            co = 2 * c2 + k
                            nc.tensor.matmul(
                                pss2[k][:],
                                w_sb["y", ci][:, co * P:(co + 1) * P],
                                xb[ci][:, cs:cs + 512], start=(ci == 0),
                                stop=(ci == CCH - 1))
                    for k in range(2):
                        co = 2 * c2 + k
                        dst = yg_t[co][:].rearrange(
                            "p (f k) -> p f k", f=NPR)[:, :, 0:P]
                        src = pss2[k][:].rearrange("p (f k) -> p f k", f=NPR)
                        nc.scalar.copy(dst, src)

            def emit_projV(xb, cs, vt_t, vt_dve):
                # PV^T (raw): per loc PAIR, [128 (2w,s), 512 co]; pair
                # chains interleaved across the two PSUM banks
                for p2 in range(NPR // 2):
                    pss2 = [pss.tile([P, 512], f32, tag="ppv", name="ppv")
                            for _ in range(2)]
                    for ci in range(CCH):
                        for k in range(2):
                            p = 2 * p2 + k
                            nc.tensor.matmul(
                                pss2[k][:],
                                xb[ci][:, cs + p * P:cs + (p + 1) * P],
                                w_sb["pv", ci][:], start=(ci == 0),
                                stop=(ci == CCH - 1))
                    for k in range(2):
                        p = 2 * p2 + k
                        if p in vt_dve:
                            with nc.allow_low_precision(
                                    reason="bf16 PV tiles fine at 2e-2"):
                                nc.vector.tensor_copy(vt_t[p][:], pss2[k][:])
                        else:
                            nc.scalar.copy(vt_t[p][:], pss2[k][:])

            def prefill_tiles(k):
                pblk, g = PREFILL[k]
                if (pblk, g) not in pf_tiles:
                    yg_t = [pfy.tile([P, YW], bf16, tag=f"pfy{k}_{co}",
                                     name=f"pfy{k}_{co}")
                            for co in range(CCH)]
                    vt_t = [pfv.tile([P, 512], bf16, tag=f"pfv{k}_{p}",
                                     name=f"pfv{k}_{p}")
                            for p in range(NPR)]
                    pf_tiles[pblk, g] = (yg_t, vt_t)
                return pf_tiles[pblk, g], g * GRP * T, xb_of[pblk]

            # ---------- phase A: stream x; stats (sum via PE ones-matmul
            # chain, sumsq via scalar/DVE Square passes); prefill on PE ----
            sacc = sc.tile([1, 512], f32, tag="sacc")
            xb_of = {}
            for blk in range(nblk):
                tiles = []
                for ci in range(CCH):
                    if blk < RES:
                        t = xres.tile([P, WT], bf16, tag=f"xr{blk}_{ci}")
                    else:
                        t = xtr.tile([P, WT], bf16, tag=f"xt{ci}")
                    nc.sync.dma_start(t[:], xs[blk, ci * P:(ci + 1) * P, :])
                    tiles.append(t)
                xb_of[blk] = tiles
                # self-contained 16-matmul column-sum chain per block (the
                # chain closes before prefill matmuls interleave), then one
                # DVE add folds it into the SBUF accumulator
                pssum = psm.tile([P, 512], f32, tag="psm")
                for ci in range(CCH):
                    xt = tiles[ci]
                    for j in range(WT // 512):
                        nc.tensor.matmul(
                            pssum[0:1, :], ocb_sb[:],
                            xt[:, j * 512:(j + 1) * 512],
                            start=(ci == 0 and j == 0),
                            stop=(ci == CCH - 1 and j == WT // 512 - 1))
                if blk == 0:
                    nc.vector.tensor_copy(sacc[:], pssum[0:1, :])
                else:
                    nc.vector.tensor_add(sacc[:], sacc[:], pssum[0:1, :])
                for ci in range(CCH):
                    i = blk * CCH + ci
                    xt = tiles[ci]
                    if ci < 2:
                        sq = sqp.tile([P, WT // 2], bf16, tag="sqs")
                        nc.scalar.activation(sq[:], xt[:, :WT // 2],
                                             AF.Square,
                                             accum_out=ssq[:, i:i + 1])
                        sq2 = sqp.tile([P, WT // 2], bf16, tag="sqs")
                        nc.scalar.activation(sq2[:], xt[:, WT // 2:],
                                             AF.Square,
                                             accum_out=ssq2[:, i:i + 1])
                    else:
                        sq = sqp.tile([P, WT // 2], bf16, tag="sqv")
                        nc.vector.scalar_tensor_tensor(
                            sq[:], xt[:, :WT // 2], 1.0, xt[:, :WT // 2],
                            ALU.mult, ALU.mult, accum_out=ssq[:, i:i + 1])
                        sq2 = sqp.tile([P, WT // 2], bf16, tag="sqv")
                        nc.vector.scalar_tensor_tensor(
                            sq2[:], xt[:, WT // 2:], 1.0, xt[:, WT // 2:],
                            ALU.mult, ALU.mult, accum_out=ssq2[:, i:i + 1])

            # ---------- stats reduce + collective ----------
            # emitted BEFORE the tail prefill groups so the reduce chain is
            # not queued behind prefill evictions that wait on the PE
            st2 = sc.tile([1, 2], f32, tag="st2")
            nc.vector.reduce_sum(out=st2[:, 0:1], in_=sacc[:], axis=AX)
            sst = sc.tile([P, 1], f32, tag="sst")
            nc.vector.tensor_add(ssq[:], ssq[:], ssq2[:])
            nc.vector.reduce_sum(out=sst[:], in_=ssq[:], axis=AX)
            sstb = sc.tile([P, 1], f32, tag="sstb")
            nc.gpsimd.partition_all_reduce(
                out_ap=sstb[:], in_ap=sst[:], channels=P,
                reduce_op=bass_isa.ReduceOp.add)
            nc.vector.tensor_copy(st2[:, 1:2], sstb[0:1, :])
            nc.gpsimd.dma_start(cc_in[:], st2[:])
            if use_collective:
                nc.gpsimd.collective_compute(
                    "AllReduce", ALU.add, replica_groups=replica_groups,
                    ins=[cc_in[:]], outs=[cc_out[:]])
            else:
                nc.gpsimd.dma_start(cc_out[:], cc_in[:])
            stg = sc.tile([1, 2], f32, tag="stg")
            nc.gpsimd.dma_start(stg[:], cc_out[:])

            # prefill groups cover the collective latency; emitted after the
            # stats-reduce chain so nothing prefill-related ever queues ahead
            # of the collective trigger (evictions go to Scalar: the DVE
            # queue is about to block on the collective-dependent v0 setup)
            for k in range(len(PREFILL)):
                pf, cs, pxb = prefill_tiles(k)
                emit_projY(pxb, cs, pf[0])
                emit_projV(pxb, cs, pf[1], vt_dve=())

            mean = sc.tile([1, 1], f32, tag="mean")
            nc.scalar.activation(mean[:], stg[:, 0:1], AF.Copy,
                                 bias=0.0, scale=1.0 / norm_n)
            ex2 = sc.tile([1, 1], f32, tag="ex2")
            nc.scalar.activation(ex2[:], stg[:, 1:2], AF.Copy,
                                 bias=0.0, scale=1.0 / norm_n)
            msq = sc.tile([1, 1], f32, tag="msq")
            nc.scalar.activation(msq[:], mean[:], AF.Square)
            varp = sc.tile([1, 1], f32, tag="varp")
            nc.vector.tensor_scalar(varp[:], ex2[:], msq[:], EPS,
                                    ALU.subtract, ALU.add)
            sqv = sc.tile([1, 1], f32, tag="sqv")      # = 1/rstd
            nc.scalar.activation(sqv[:], varp[:], AF.Sqrt)
            rst = sc.tile([1, 1], f32, tag="rst")      # = rstd
            nc.vector.reciprocal(rst[:], sqv[:])
            rmu = sc.tile([1, 1], f32, tag="rmu")      # = rstd*mean
            nc.vector.tensor_scalar(rmu[:], mean[:], rst[:], None, ALU.mult)
            rsq = sc.tile([1, 1], f32, tag="rsq")      # = rstd^2
            nc.vector.tensor_scalar(rsq[:], rst[:], rst[:], None, ALU.mult)
            vals = sc.tile([1, 4], f32, tag="vals")
            nc.vector.tensor_copy(vals[:, 0:1], rst[:])
            nc.vector.tensor_copy(vals[:, 1:2], rmu[:])
            nc.vector.tensor_copy(vals[:, 2:3], sqv[:])
            nc.vector.tensor_copy(vals[:, 3:4], rsq[:])
            # broadcast (rstd, rstd*mean, 1/rstd, rstd^2) across partitions
            rb = sc.tile([P, 4], f32, tag="rb")
            nc.gpsimd.partition_broadcast(rb[:], vals[:])
            # all-(1/r) stationary for the softmax denominator matmul: the
            # rowsum matmul then directly yields Z/r, whose reciprocal r/Z
            # is folded into the attention weights before AV
            oiv = sc.tile([P, P], bf16, tag="oiv")
            nc.vector.tensor_scalar(oiv[:], omb_sb[:], rb[:, 2:3], None,
                                    ALU.mult)
            # score rank-1 vector, pre-scaled so the PSUM h-column can feed
            # the exp bias directly: bias must be r*(w1 - rmu*w2)^T x
            v0c = sc.tile([P, CCH], f32, tag="v0c")
            nc.vector.tensor_scalar(v0c[:], w2_sb[:], rb[:, 1:2], None,
                                    ALU.mult)
            nc.vector.tensor_sub(v0c[:], w1_sb[:], v0c[:])
            v0b = sc.tile([P, CCH], bf16, tag="v0b")
            nc.vector.tensor_scalar(v0b[:], v0c[:], rb[:, 0:1], None,
                                    ALU.mult)
            # v0 replicated 4x per co-chunk so one strided DVE copy fills
            # the 4 pair-columns of a yg tile
            v0b4 = sc.tile([P, CCH * NPR], bf16, tag="v0b4")
            for j in range(NPR):
                nc.vector.tensor_copy(
                    v0b4[:].rearrange("p (c j) -> p c j", j=NPR)[:, :, j:j + 1],
                    v0b[:].rearrange("p (c o) -> p c o", o=1))
            # dp = Wp @ dv + bp = pv1 - rmu*pv2 (host-folded vectors)
            dp = sc.tile([P, CCH], f32, tag="dp")
            nc.vector.tensor_scalar(dp[:], pv2_sb[:], rb[:, 1:2], None,
                                    ALU.mult)
            nc.vector.tensor_sub(dp[:], pv1_sb[:], dp[:])

            # ---------- main loop ----------
            def emit_v0cols(yg_t):
                for co in range(CCH):
                    dst = yg_t[co][:].rearrange(
                        "p (f k) -> p f k", f=NPR)[:, :, P:P + 1]
                    src = v0b4[:, co * NPR:(co + 1) * NPR].rearrange(
                        "p (j o) -> p j o", o=1)
                    nc.vector.tensor_copy(dst, src)

            def emit_scores(xb, cs, yg_t):
                sc01 = scp.tile([P, 2 * (P + 1)], f32, tag="sc01")
                sc23 = scp.tile([P, 2 * (P + 1)], f32, tag="sc23")
                for p in range(NPR):
                    dst = sc01 if p < 2 else sc23
                    off = (p % 2) * (P + 1)
                    for ci in range(CCH):
                        nc.tensor.matmul(
                            dst[:, off:off + P + 1],
                            xb[ci][:, cs + p * P:cs + (p + 1) * P],
                            yg_t[ci][:, p * (P + 1):(p + 1) * (P + 1)],
                            start=(p % 2 == 0 and ci == 0),
                            stop=(p % 2 == 1 and ci == CCH - 1),
                            skip_group_check=True)
                return sc01, sc23

            def softmax_head(st):
                # exp with per-partition bias read straight from the PSUM
                # h-column; then causal+pair mask
                _blk, _g, _xb, _cs, _vt, sc01, sc23 = st
                # h-columns PSUM -> SBUF (activation bias must be SBUF)
                hb = spool.tile([P, NPR], f32, tag="hb")
                for i, bank in enumerate((sc01, sc23)):
                    nc.vector.tensor_copy(
                        hb[:, 2 * i:2 * i + 2].rearrange(
                            "p (f o) -> p f o", o=1),
                        bank[:].rearrange("p (f k) -> p f k", f=2)
                        [:, :, P:P + 1])
                pexp = spool.tile([P, 512], bf16, tag="pexp")
                for p in range(NPR):
                    src = sc01 if p < 2 else sc23
                    off = (p % 2) * (P + 1)
                    nc.scalar.activation(
                        pexp[:, p * P:(p + 1) * P], src[:, off:off + P],
                        AF.Exp, scale=rb[:, 3:4],
                        bias=hb[:, p:p + 1])
                pm = spool.tile([P, 512], bf16, tag="pm")
                nc.vector.tensor_mul(pm[:], pexp[:], mask_sb[:])
                return pm

            def rowsum_part(pm):
                ps_z = psm.tile([P, 512], f32, tag="psm")
                nc.tensor.matmul(ps_z[:], oiv[:], pm[:], start=True,
                                 stop=True)
                rz = rzp.tile([P, 512], f32, tag="rz")
                nc.vector.reciprocal_approx_fast(out=rz[:], in_=ps_z[:])
                pmn = spool.tile([P, 512], bf16, tag="pmn")
                nc.vector.tensor_mul(pmn[:], pm[:], rz[:])
                return pmn

            def av_tail(st, pmn):
                blk, _g, xb, cs, vt, _s1, _s2 = st
                for ch in range(CCH):
                    ps_o = pp.tile([P, 512], f32, tag="pp")
                    for p in range(NPR):
                        nc.tensor.matmul(
                            ps_o[:, p * P:(p + 1) * P],
                            vt[p][:, ch * P:(ch + 1) * P],
                            pmn[:, p * P:(p + 1) * P],
                            start=(p == 0), stop=True,
                            skip_group_check=True)
                    slab = opool.tile([P, 512], f32, tag="oslab")
                    nc.vector.scalar_tensor_tensor(
                        slab[:], ps_o[:], dp[:, ch:ch + 1],
                        xb[ch][:, cs:cs + 512], ALU.add, ALU.add)
                    nc.sync.dma_start(
                        outp[blk, ch * P:(ch + 1) * P, cs:cs + 512],
                        slab[:])

            def redma(blk):
                tiles = []
                for ci in range(CCH):
                    t = xtr.tile([P, WT], bf16, tag=f"xt{ci}", name=f"xt{ci}")
                    nc.sync.dma_start(t[:], xs[blk, ci * P:(ci + 1) * P, :])
                    tiles.append(t)
                xb_of[blk] = tiles

            # transient blocks were overwritten during the stats pass;
            # stream them back in well ahead of use
            redma(RES)
            redma(RES + 1)

            # order: four full groups lead (their projections keep the PE
            # fed through the collective's launch-skew tail); then prefilled
            # (cheap, score-only) groups alternate with full groups, so the
            # softmax latency of a cheap group hides under a full group's
            # matmuls
            NLEAD = 4
            cheap = sorted(pf_tiles)
            full = [(blk, g) for blk in range(RES)
                    for g in range(NGRP) if (blk, g) not in pf_tiles]
            order = full[:NLEAD]
            for i, c in enumerate(cheap):
                order.append(c)
                if NLEAD + i < len(full):
                    order.append(full[NLEAD + i])
            order += full[NLEAD + len(cheap):]
            for blk in range(RES, nblk):
                for g in range(NGRP):
                    order.append((blk, g))
            assert len(order) == nblk * NGRP and len(set(order)) == len(order)

            pend_sm = None        # awaiting softmax head + rowsum (g-1)
            pend_av = None        # awaiting AV + eviction (g-2)
            pend_pmn = None
            for it, (blk, g) in enumerate(order):
                xb = xb_of[blk]
                cs = g * GRP * T
                pm = softmax_head(pend_sm) if pend_sm is not None else None
                if (blk, g) in pf_tiles:
                    yg_t, vt_t = pf_tiles[blk, g]
                else:
                    yg_t = [ygp.tile([P, YW], bf16, tag="yg", name="yg")
                            for _ in range(CCH)]
                    vt_t = [vtpool.tile([P, 512], bf16, tag="vt", name="vt")
                            for _ in range(NPR)]
                    emit_projY(xb, cs, yg_t)
                    # early iterations: all evictions on Scalar — the DVE
                    # queue is blocked on the collective-dependent v0 setup
                    # and would stall the PSUM rotation
                    emit_projV(xb, cs, vt_t,
                               vt_dve=(2, 3) if it >= 12 else ())
                emit_v0cols(yg_t)
                sc01, sc23 = emit_scores(xb, cs, yg_t)
                nxt_pmn = rowsum_part(pm) if pm is not None else None
                if pend_av is not None:
                    av_tail(pend_av, pend_pmn)
                pend_av, pend_pmn = pend_sm, nxt_pmn
                pend_sm = (blk, g, xb, cs, vt_t, sc01, sc23)
                if (blk, g) == (4, NGRP - 1):
                    redma(6)
                elif (blk, g) == (5, NGRP - 1):
                    redma(7)
            # drain the two in-flight groups (g-2 first so its AV runs
            # while g-1's softmax finishes)
            pm = softmax_head(pend_sm)
            av_tail(pend_av, pend_pmn)
            nxt_pmn = rowsum_part(pm)
            av_tail(pend_sm, nxt_pmn)
    nc.compile()
    return nc


def host_prep(gamma, beta, wq, bq, wk, bk, wv, bv, wp, bp):
    """Fold gamma/beta into weights; build all constant tensors."""
    s = 1.0 / np.sqrt(np.float64(C))
    g = gamma.astype(np.float64)

    def fold(w, bias, scale):
        a = (w.astype(np.float64) * g[None, :]) * scale      # (co, ci)
        u = (w.astype(np.float64) @ g) * scale               # (co,)
        c0 = (bias.astype(np.float64) + w.astype(np.float64) @
              beta.astype(np.float64)) * scale
        return a, u, c0

    aq, uq, cq = fold(wq, bq, s)
    ak, uk, ck = fold(wk, bk, 1.0)
    av, uv, cv = fold(wv, bv, 1.0)
    # scores are bilinear: S = (Ak x)^T (Aq x) = x^T G x with G = Ak^T Aq;
    # the surviving affine term (s-dependent only — t-terms cancel in
    # softmax) uses w1/w2: h = x^T Ak^T (cq - mu*r*uq)
    G = ak.T @ aq
    w1 = ak.T @ cq
    w2 = ak.T @ uq
    # output projection folded into V: Apv = Wp @ Av; P-eviction constant
    # dp = Wp@(cv - mu*r*uv) + bp = pv1 - mu*r*pv2
    wp64 = wp.astype(np.float64)
    apv = wp64 @ av
    pv1 = wp64 @ cv + bp.astype(np.float64)
    pv2 = wp64 @ uv
    gyt = np.ascontiguousarray(G.T).astype(BF)
    apvt = np.ascontiguousarray(apv.T).astype(BF)

    def colize(v):
        out = np.empty((P, CCH), np.float32)
        for ch in range(CCH):
            out[:, ch] = v[ch * P:(ch + 1) * P]
        return out

    w1c = colize(w1)
    w2c = colize(w2)
    pv1c = colize(pv1)
    pv2c = colize(pv2)

    # pair mask [128, 4*128]: diag 64x64 halves get causal triu (s<=t),
    # off-diag (cross-location) halves are zero; identical per pair.
    tri = np.triu(np.ones((T, T), np.float32))
    blkm = np.zeros((P, P), np.float32)
    blkm[0:T, 0:T] = tri
    blkm[T:2 * T, T:2 * T] = tri
    maskt = np.tile(blkm, (1, NPR))

    consts = {
        "wyt": gyt, "wpvt": apvt,
        "w1col": w1c, "w2col": w2c, "pv1col": pv1c, "pv2col": pv2c,
        "maskt": maskt.astype(BF),
        "ones_mat_b": np.ones((P, P), BF),
        "ones_col_b": np.ones((P, 1), BF),
    }
    return consts


_NC_CACHE = {}


def kernel(x, gamma, beta, wq, bq, wk, bk, wv, bv, wp, bp):
    x = np.asarray(x, np.float32)
    args = [np.asarray(a, np.float32) for a in
            (gamma, beta, wq, bq, wk, bk, wv, bv, wp, bp)]
    consts = host_prep(*args)

    if "nc" not in _NC_CACHE:
        _NC_CACHE["nc"] = build_nc()
    nc = _NC_CACHE["nc"]

    in_maps = []
    for core in range(NCORES):
        b, hg = core // 4, core % 4
        shard = x[b, :, :, hg * HSH:(hg + 1) * HSH, :]        # (C,T,HSH,W)
        shard = np.ascontiguousarray(
            shard.transpose(2, 0, 3, 1)).reshape(HSH, C, WT)  # w-major
        in_maps.append({"xs": shard.astype(BF), **consts})

    global _last_in_maps
    _last_in_maps = in_maps
    res = run_bass_kernel_spmd(nc, in_maps, list(range(NCORES)))

    out = np.empty((B, C, T, H, W), np.float32)
    for core in range(NCORES):
        b, hg = core // 4, core % 4
        o = res.results[core]["out"].reshape(HSH, C, W, T)
        out[b, :, :, hg * HSH:(hg + 1) * HSH, :] = o.transpose(1, 3, 0, 2)
    return out


# revision 35
# speedup vs baseline: 1.0731x; 1.0731x over previous
"""CausalTemporalAttnBlock Trainium2 kernel.

Problem: out = x + Wp @ attn(norm(x)) + bp, where norm is GroupNorm(1 group)
over (c,t,h,w) per batch, attention is causal over t, independent per (b,h,w).
Shapes: x (2, 512, 64, 32, 32) fp32; four (512,512) weights + biases.

Strategy (8 NeuronCores, zero communication except an 8-byte AllReduce for
the GroupNorm stats):
  - core i handles batch i//4, h-rows [8*(i%4), 8*(i%4)+8), all w: 256 (h,w)
    locations per core. Host re-lays the shard w-major:
    [8 h-rows][512 c][32 w * 64 t], so one attention group (8 w-locations)
    is a contiguous 512-column slice. Whole matmul datapath bf16 (fp32 PSUM).
  - Channel mixing commutes with temporal attention, so the output
    projection folds into the V projection on the host: Apv = Wp @ Av and
        out = x + r * Apv (x p^T) + dp,   dp = Wp dv + bp
    (softmax rows sum to 1, so the V-path affine passes through attention
    as a per-channel constant). This deletes a full (512,512) projection.
  - Q/K fold into one bilinear form: scores = x^T G x with G = Ak^T Aq,
    so a single Y = G x projection replaces both Q and K.
  - The surviving score affine term h[s] = x^T v0 rides as a 129th moving
    column in each score matmul (v0 appended after each pair's 128 t-cols),
    landing in PSUM next to the scores; the Exp activation consumes it as
    a per-partition bias AP. No dedicated PE work for it at all.
  - Locations are processed in PAIRS sharing the 128-wide stationary
    operand: PV^T for 2 locations as one matmul chain, scores S^T per pair
    as [128,129] chains (cross terms masked off), AV as full-K [128,128]
    matmuls (masked zeros in the normalized weights kill cross terms).
  - GroupNorm stats run entirely OFF the PE (DVE reduce for sums;
    Square/sumsq split across Scalar, GpSimd and DVE). While the stats
    pass streams x from HBM, the PE prefills the first 6 groups'
    Y/PV projections, so the collective latency hides under matmuls.
  - Softmax: no max-subtraction (scores are O(1)); exp applies the r^2
    the bilinear form is missing; normalization r/Z is multiplied into
    the masked exp weights (bf16) before the AV matmuls, so the AV
    eviction is a single fused (psum + dp) + x DVE op per chunk.
"""

import numpy as np
import ml_dtypes

import concourse.bass as bass
import concourse.tile as tile
from concourse import bacc, bass_isa, mybir
from concourse.bass_utils import run_bass_kernel_spmd

P = 128
B, C, T, H, W = 2, 512, 64, 32, 32
NCORES = 8
HSH = H // 4          # 8 h-rows per core
CCH = C // P          # 4 c chunks
GRP = 8               # locations per attention group
NGRP = W // GRP       # 4 groups per h-row block
NPR = GRP // 2        # 4 location-pairs per group
WT = W * T            # 2048 free columns per (h-row, c) plane
YW = NPR * (P + 1)    # 516: yg tile width (4 pair-blocks of 128 t-cols + v0)
RES = 4               # h-row blocks kept resident in SBUF
EPS = 1e-6

f32 = mybir.dt.float32
bf16 = mybir.dt.bfloat16
AX = mybir.AxisListType.X
ALU = mybir.AluOpType
AF = mybir.ActivationFunctionType
BF = ml_dtypes.bfloat16

# groups prefilled (Y/PV projections) during the stats pass, as (blk, g)
PREFILL = [(0, 0), (0, 1), (0, 2), (0, 3), (1, 0), (1, 1)]


def build_nc(num_cores=NCORES, nblk=HSH, norm_n=None, replica_groups=None,
             use_collective=True):
    if norm_n is None:
        norm_n = C * T * H * W
    if replica_groups is None:
        replica_groups = [[0, 1, 2, 3], [4, 5, 6, 7]]
    nc = bacc.Bacc("TRN2", target_bir_lowering=False, debug=False,
                   num_devices=num_cores)

    xs = nc.declare_dram_parameter("xs", [nblk, C, WT], bf16, isOutput=False)
    wts = {}
    for nm in ("y", "pv"):
        wts[nm] = nc.declare_dram_parameter(f"w{nm}t", [C, C], bf16,
                                            isOutput=False)
    w1col = nc.declare_dram_parameter("w1col", [P, CCH], f32, isOutput=False)
    w2col = nc.declare_dram_parameter("w2col", [P, CCH], f32, isOutput=False)
    pv1col = nc.declare_dram_parameter("pv1col", [P, CCH], f32, isOutput=False)
    pv2col = nc.declare_dram_parameter("pv2col", [P, CCH], f32, isOutput=False)
    maskp = nc.declare_dram_parameter("maskt", [P, NPR * P], bf16,
                                      isOutput=False)
    ones_mat_b = nc.declare_dram_parameter("ones_mat_b", [P, P], bf16,
                                           isOutput=False)
    ones_col_b = nc.declare_dram_parameter("ones_col_b", [P, 1], bf16,
                                           isOutput=False)
    outp = nc.declare_dram_parameter("out", [nblk, C, WT], f32, isOutput=True)
    cc_in = nc.dram_tensor("cc_in", [1, 2], f32)
    cc_out = nc.dram_tensor("cc_out", [1, 2], f32)

    with tile.TileContext(nc) as tc:
        with (
            tc.tile_pool(name="const", bufs=1) as const,
            tc.tile_pool(name="scal", bufs=1) as sc,
            tc.tile_pool(name="xres", bufs=1) as xres,
            tc.tile_pool(name="xtr", bufs=2) as xtr,
            tc.tile_pool(name="sqp", bufs=1) as sqp,
            tc.tile_pool(name="pfy", bufs=1) as pfy,
            tc.tile_pool(name="pfv", bufs=1) as pfv,
            tc.tile_pool(name="ygp", bufs=12) as ygp,
            tc.tile_pool(name="vtp", bufs=16) as vtpool,
            tc.tile_pool(name="spool", bufs=2) as spool,
            tc.tile_pool(name="rzp", bufs=2) as rzp,
            tc.tile_pool(name="opool", bufs=3) as opool,
            tc.tile_pool(name="pp", bufs=3, space="PSUM") as pp,
            tc.tile_pool(name="pss", bufs=2, space="PSUM") as pss,
            tc.tile_pool(name="scp", bufs=1, space="PSUM") as scp,
            tc.tile_pool(name="psm", bufs=1, space="PSUM") as psm,
        ):
            # ---------- constants ----------
            w_sb = {}
            for nm in ("y", "pv"):
                for ci in range(CCH):
                    t = const.tile([P, C], bf16, tag=f"w{nm}{ci}")
                    nc.sync.dma_start(t[:], wts[nm][ci * P:(ci + 1) * P, :])
                    w_sb[nm, ci] = t
            w1_sb = const.tile([P, CCH], f32, tag="w1col")
            nc.sync.dma_start(w1_sb[:], w1col[:])
            w2_sb = const.tile([P, CCH], f32, tag="w2col")
            nc.sync.dma_start(w2_sb[:], w2col[:])
            pv1_sb = const.tile([P, CCH], f32, tag="pv1col")
            nc.sync.dma_start(pv1_sb[:], pv1col[:])
            pv2_sb = const.tile([P, CCH], f32, tag="pv2col")
            nc.sync.dma_start(pv2_sb[:], pv2col[:])
            mask_sb = const.tile([P, NPR * P], bf16, tag="maskt")
            nc.sync.dma_start(mask_sb[:], maskp[:])
            omb_sb = const.tile([P, P], bf16, tag="omb")
            nc.sync.dma_start(omb_sb[:], ones_mat_b[:])
            ocb_sb = const.tile([P, 1], bf16, tag="ocb")
            nc.sync.dma_start(ocb_sb[:], ones_col_b[:])

            ssq = sc.tile([P, nblk * CCH], f32, tag="ssq")
            ssq2 = sc.tile([P, nblk * CCH], f32, tag="ssq2")

            # prefill projection outputs, keyed (blk, g)
            pf_tiles = {}

            def emit_projY(xb, cs, yg_t):
                # Y = G^T x: per co-chunk, chain over ci; evict to the
                # 4x129-strided yg layout (col 128 of each pair block is
                # written later with v0)
                for co in range(CCH):
                    ps = pp.tile([P, 512], f32, tag="pp")
                    for ci in range(CCH):
                        nc.tensor.matmul(
                            ps[:], w_sb["y", ci][:, co * P:(co + 1) * P],
                            xb[ci][:, cs:cs + 512], start=(ci == 0),
                            stop=(ci == CCH - 1))
                    dst = yg_t[co][:].rearrange(
                        "p (f k) -> p f k", f=NPR)[:, :, 0:P]
                    src = ps[:].rearrange("p (f k) -> p f k", f=NPR)
                    nc.scalar.copy(dst, src)

            def emit_projV(xb, cs, vt_t, vt_dve):
                # PV^T (raw): per loc PAIR, [128 (2w,s), 512 co]
                for p in range(NPR):
                    ps = pss.tile([P, 512], f32, tag="ppv")
                    for ci in range(CCH):
                        nc.tensor.matmul(
                            ps[:], xb[ci][:, cs + p * P:cs + (p + 1) * P],
                            w_sb["pv", ci][:], start=(ci == 0),
                            stop=(ci == CCH - 1))
                    if p in vt_dve:
                        with nc.allow_low_precision(
                                reason="bf16 PV tiles fine at 2e-2 target"):
                            nc.vector.tensor_copy(vt_t[p][:], ps[:])
                    else:
                        nc.scalar.copy(vt_t[p][:], ps[:])

            def prefill_tiles(k):
                pblk, g = PREFILL[k]
                if (pblk, g) not in pf_tiles:
                    yg_t = [pfy.tile([P, YW], bf16, tag=f"pfy{k}_{co}",
                                     name=f"pfy{k}_{co}")
                            for co in range(CCH)]
                    vt_t = [pfv.tile([P, 512], bf16, tag=f"pfv{k}_{p}",
                                     name=f"pfv{k}_{p}")
                            for p in range(NPR)]
                    pf_tiles[pblk, g] = (yg_t, vt_t)
                return pf_tiles[pblk, g], g * GRP * T, xb_of[pblk]

            # ---------- phase A: stream x; stats (sum via PE ones-matmul
            # chain, sumsq via scalar/DVE Square passes); prefill on PE ----
            sacc = sc.tile([1, 512], f32, tag="sacc")
            xb_of = {}
            for blk in range(nblk):
                tiles = []
                for ci in range(CCH):
                    if blk < RES:
                        t = xres.tile([P, WT], bf16, tag=f"xr{blk}_{ci}")
                    else:
                        t = xtr.tile([P, WT], bf16, tag=f"xt{ci}")
                    nc.sync.dma_start(t[:], xs[blk, ci * P:(ci + 1) * P, :])
                    tiles.append(t)
                xb_of[blk] = tiles
                # self-contained 16-matmul column-sum chain per block (the
                # chain closes before prefill matmuls interleave), then one
                # DVE add folds it into the SBUF accumulator
                pssum = psm.tile([P, 512], f32, tag="psm")
                for ci in range(CCH):
                    xt = tiles[ci]
                    for j in range(WT // 512):
                        nc.tensor.matmul(
                            pssum[0:1, :], ocb_sb[:],
                            xt[:, j * 512:(j + 1) * 512],
                            start=(ci == 0 and j == 0),
                            stop=(ci == CCH - 1 and j == WT // 512 - 1))
                if blk == 0:
                    nc.vector.tensor_copy(sacc[:], pssum[0:1, :])
                else:
                    nc.vector.tensor_add(sacc[:], sacc[:], pssum[0:1, :])
                for ci in range(CCH):
                    i = blk * CCH + ci
                    xt = tiles[ci]
                    if ci < 2:
                        sq = sqp.tile([P, WT // 2], bf16, tag="sqs")
                        nc.scalar.activation(sq[:], xt[:, :WT // 2],
                                             AF.Square,
                                             accum_out=ssq[:, i:i + 1])
                        sq2 = sqp.tile([P, WT // 2], bf16, tag="sqs")
                        nc.scalar.activation(sq2[:], xt[:, WT // 2:],
                                             AF.Square,
                                             accum_out=ssq2[:, i:i + 1])
                    else:
                        sq = sqp.tile([P, WT // 2], bf16, tag="sqv")
                        nc.vector.scalar_tensor_tensor(
                            sq[:], xt[:, :WT // 2], 1.0, xt[:, :WT // 2],
                            ALU.mult, ALU.mult, accum_out=ssq[:, i:i + 1])
                        sq2 = sqp.tile([P, WT // 2], bf16, tag="sqv")
                        nc.vector.scalar_tensor_tensor(
                            sq2[:], xt[:, WT // 2:], 1.0, xt[:, WT // 2:],
                            ALU.mult, ALU.mult, accum_out=ssq2[:, i:i + 1])

            # ---------- stats reduce + collective ----------
            # emitted BEFORE the tail prefill groups so the reduce chain is
            # not queued behind prefill evictions that wait on the PE
            st2 = sc.tile([1, 2], f32, tag="st2")
            nc.vector.reduce_sum(out=st2[:, 0:1], in_=sacc[:], axis=AX)
            sst = sc.tile([P, 1], f32, tag="sst")
            nc.vector.tensor_add(ssq[:], ssq[:], ssq2[:])
            nc.vector.reduce_sum(out=sst[:], in_=ssq[:], axis=AX)
            sstb = sc.tile([P, 1], f32, tag="sstb")
            nc.gpsimd.partition_all_reduce(
                out_ap=sstb[:], in_ap=sst[:], channels=P,
                reduce_op=bass_isa.ReduceOp.add)
            nc.vector.tensor_copy(st2[:, 1:2], sstb[0:1, :])
            nc.gpsimd.dma_start(cc_in[:], st2[:])
            if use_collective:
                nc.gpsimd.collective_compute(
                    "AllReduce", ALU.add, replica_groups=replica_groups,
                    ins=[cc_in[:]], outs=[cc_out[:]])
            else:
                nc.gpsimd.dma_start(cc_out[:], cc_in[:])
            stg = sc.tile([1, 2], f32, tag="stg")
            nc.gpsimd.dma_start(stg[:], cc_out[:])

            # prefill groups cover the collective latency; emitted after the
            # stats-reduce chain so nothing prefill-related ever queues ahead
            # of the collective trigger (evictions go to Scalar: the DVE
            # queue is about to block on the collective-dependent v0 setup)
            for k in range(len(PREFILL)):
                pf, cs, pxb = prefill_tiles(k)
                emit_projY(pxb, cs, pf[0])
                emit_projV(pxb, cs, pf[1], vt_dve=())

            mean = sc.tile([1, 1], f32, tag="mean")
            nc.scalar.activation(mean[:], stg[:, 0:1], AF.Copy,
                                 bias=0.0, scale=1.0 / norm_n)
            ex2 = sc.tile([1, 1], f32, tag="ex2")
            nc.scalar.activation(ex2[:], stg[:, 1:2], AF.Copy,
                                 bias=0.0, scale=1.0 / norm_n)
            msq = sc.tile([1, 1], f32, tag="msq")
            nc.scalar.activation(msq[:], mean[:], AF.Square)
            varp = sc.tile([1, 1], f32, tag="varp")
            nc.vector.tensor_scalar(varp[:], ex2[:], msq[:], EPS,
                                    ALU.subtract, ALU.add)
            sqv = sc.tile([1, 1], f32, tag="sqv")      # = 1/rstd
            nc.scalar.activation(sqv[:], varp[:], AF.Sqrt)
            rst = sc.tile([1, 1], f32, tag="rst")      # = rstd
            nc.vector.reciprocal(rst[:], sqv[:])
            rmu = sc.tile([1, 1], f32, tag="rmu")      # = rstd*mean
            nc.vector.tensor_scalar(rmu[:], mean[:], rst[:], None, ALU.mult)
            rsq = sc.tile([1, 1], f32, tag="rsq")      # = rstd^2
            nc.vector.tensor_scalar(rsq[:], rst[:], rst[:], None, ALU.mult)
            vals = sc.tile([1, 4], f32, tag="vals")
            nc.vector.tensor_copy(vals[:, 0:1], rst[:])
            nc.vector.tensor_copy(vals[:, 1:2], rmu[:])
            nc.vector.tensor_copy(vals[:, 2:3], sqv[:])
            nc.vector.tensor_copy(vals[:, 3:4], rsq[:])
            # broadcast (rstd, rstd*mean, 1/rstd, rstd^2) across partitions
            rb = sc.tile([P, 4], f32, tag="rb")
            nc.gpsimd.partition_broadcast(rb[:], vals[:])
            # all-(1/r) stationary for the softmax denominator matmul: the
            # rowsum matmul then directly yields Z/r, whose reciprocal r/Z
            # is folded into the attention weights before AV
            oiv = sc.tile([P, P], bf16, tag="oiv")
            nc.vector.tensor_scalar(oiv[:], omb_sb[:], rb[:, 2:3], None,
                                    ALU.mult)
            # score rank-1 vector, pre-scaled so the PSUM h-column can feed
            # the exp bias directly: bias must be r*(w1 - rmu*w2)^T x
            v0c = sc.tile([P, CCH], f32, tag="v0c")
            nc.vector.tensor_scalar(v0c[:], w2_sb[:], rb[:, 1:2], None,
                                    ALU.mult)
            nc.vector.tensor_sub(v0c[:], w1_sb[:], v0c[:])
            v0b = sc.tile([P, CCH], bf16, tag="v0b")
            nc.vector.tensor_scalar(v0b[:], v0c[:], rb[:, 0:1], None,
                                    ALU.mult)
            # v0 replicated 4x per co-chunk so one strided DVE copy fills
            # the 4 pair-columns of a yg tile
            v0b4 = sc.tile([P, CCH * NPR], bf16, tag="v0b4")
            for j in range(NPR):
                nc.vector.tensor_copy(
                    v0b4[:].rearrange("p (c j) -> p c j", j=NPR)[:, :, j:j + 1],
                    v0b[:].rearrange("p (c o) -> p c o", o=1))
            # dp = Wp @ dv + bp = pv1 - rmu*pv2 (host-folded vectors)
            dp = sc.tile([P, CCH], f32, tag="dp")
            nc.vector.tensor_scalar(dp[:], pv2_sb[:], rb[:, 1:2], None,
                                    ALU.mult)
            nc.vector.tensor_sub(dp[:], pv1_sb[:], dp[:])

            # ---------- main loop ----------
            def emit_v0cols(yg_t):
                for co in range(CCH):
                    dst = yg_t[co][:].rearrange(
                        "p (f k) -> p f k", f=NPR)[:, :, P:P + 1]
                    src = v0b4[:, co * NPR:(co + 1) * NPR].rearrange(
                        "p (j o) -> p j o", o=1)
                    nc.vector.tensor_copy(dst, src)

            def emit_scores(xb, cs, yg_t):
                sc01 = scp.tile([P, 2 * (P + 1)], f32, tag="sc01")
                sc23 = scp.tile([P, 2 * (P + 1)], f32, tag="sc23")
                for p in range(NPR):
                    dst = sc01 if p < 2 else sc23
                    off = (p % 2) * (P + 1)
                    for ci in range(CCH):
                        nc.tensor.matmul(
                            dst[:, off:off + P + 1],
                            xb[ci][:, cs + p * P:cs + (p + 1) * P],
                            yg_t[ci][:, p * (P + 1):(p + 1) * (P + 1)],
                            start=(p % 2 == 0 and ci == 0),
                            stop=(p % 2 == 1 and ci == CCH - 1),
                            skip_group_check=True)
                return sc01, sc23

            def softmax_head(st):
                # exp with per-partition bias read straight from the PSUM
                # h-column; then causal+pair mask
                _blk, _g, _xb, _cs, _vt, sc01, sc23 = st
                # h-columns PSUM -> SBUF (activation bias must be SBUF)
                hb = spool.tile([P, NPR], f32, tag="hb")
                for i, bank in enumerate((sc01, sc23)):
                    nc.vector.tensor_copy(
                        hb[:, 2 * i:2 * i + 2].rearrange(
                            "p (f o) -> p f o", o=1),
                        bank[:].rearrange("p (f k) -> p f k", f=2)
                        [:, :, P:P + 1])
                pexp = spool.tile([P, 512], bf16, tag="pexp")
                for p in range(NPR):
                    src = sc01 if p < 2 else sc23
                    off = (p % 2) * (P + 1)
                    nc.scalar.activation(
                        pexp[:, p * P:(p + 1) * P], src[:, off:off + P],
                        AF.Exp, scale=rb[:, 3:4],
                        bias=hb[:, p:p + 1])
                pm = spool.tile([P, 512], bf16, tag="pm")
                nc.vector.tensor_mul(pm[:], pexp[:], mask_sb[:])
                return pm

            def rowsum_part(pm):
                ps_z = psm.tile([P, 512], f32, tag="psm")
                nc.tensor.matmul(ps_z[:], oiv[:], pm[:], start=True,
                                 stop=True)
                rz = rzp.tile([P, 512], f32, tag="rz")
                nc.vector.reciprocal_approx_fast(out=rz[:], in_=ps_z[:])
                pmn = spool.tile([P, 512], bf16, tag="pmn")
                nc.vector.tensor_mul(pmn[:], pm[:], rz[:])
                return pmn

            def av_tail(st, pmn):
                blk, _g, xb, cs, vt, _s1, _s2 = st
                for ch in range(CCH):
                    ps_o = pp.tile([P, 512], f32, tag="pp")
                    for p in range(NPR):
                        nc.tensor.matmul(
                            ps_o[:, p * P:(p + 1) * P],
                            vt[p][:, ch * P:(ch + 1) * P],
                            pmn[:, p * P:(p + 1) * P],
                            start=(p == 0), stop=True,
                            skip_group_check=True)
                    slab = opool.tile([P, 512], f32, tag="oslab")
                    nc.vector.scalar_tensor_tensor(
                        slab[:], ps_o[:], dp[:, ch:ch + 1],
                        xb[ch][:, cs:cs + 512], ALU.add, ALU.add)
                    nc.sync.dma_start(
                        outp[blk, ch * P:(ch + 1) * P, cs:cs + 512],
                        slab[:])

            def redma(blk):
                tiles = []
                for ci in range(CCH):
                    t = xtr.tile([P, WT], bf16, tag=f"xt{ci}", name=f"xt{ci}")
                    nc.sync.dma_start(t[:], xs[blk, ci * P:(ci + 1) * P, :])
                    tiles.append(t)
                xb_of[blk] = tiles

            # transient blocks were overwritten during the stats pass;
            # stream them back in well ahead of use
            redma(RES)
            redma(RES + 1)

            # order: four full groups lead (their projections keep the PE
            # fed through the collective's launch-skew tail); then prefilled
            # (cheap, score-only) groups alternate with full groups, so the
            # softmax latency of a cheap group hides under a full group's
            # matmuls
            NLEAD = 4
            cheap = sorted(pf_tiles)
            full = [(blk, g) for blk in range(RES)
                    for g in range(NGRP) if (blk, g) not in pf_tiles]
            order = full[:NLEAD]
            for i, c in enumerate(cheap):
                order.append(c)
                if NLEAD + i < len(full):
                    order.append(full[NLEAD + i])
            order += full[NLEAD + len(cheap):]
            for blk in range(RES, nblk):
                for g in range(NGRP):
                    order.append((blk, g))
            assert len(order) == nblk * NGRP and len(set(order)) == len(order)

            pend_sm = None        # awaiting softmax head + rowsum (g-1)
            pend_av = None        # awaiting AV + eviction (g-2)
            pend_pmn = None
            for it, (blk, g) in enumerate(order):
                xb = xb_of[blk]
                cs = g * GRP * T
                pm = softmax_head(pend_sm) if pend_sm is not None else None
                if (blk, g) in pf_tiles:
                    yg_t, vt_t = pf_tiles[blk, g]
                else:
                    yg_t = [ygp.tile([P, YW], bf16, tag="yg", name="yg")
                            for _ in range(CCH)]
                    vt_t = [vtpool.tile([P, 512], bf16, tag="vt", name="vt")
                            for _ in range(NPR)]
                    emit_projY(xb, cs, yg_t)
                    # early iterations: all evictions on Scalar — the DVE
                    # queue is blocked on the collective-dependent v0 setup
                    # and would stall the PSUM rotation
                    emit_projV(xb, cs, vt_t,
                               vt_dve=(2, 3) if it >= 12 else ())
                emit_v0cols(yg_t)
                sc01, sc23 = emit_scores(xb, cs, yg_t)
                nxt_pmn = rowsum_part(pm) if pm is not None else None
                if pend_av is not None:
                    av_tail(pend_av, pend_pmn)
                pend_av, pend_pmn = pend_sm, nxt_pmn
                pend_sm = (blk, g, xb, cs, vt_t, sc01, sc23)
                if (blk, g) == (4, NGRP - 1):
                    redma(6)
                elif (blk, g) == (5, NGRP - 1):
                    redma(7)
            # drain the two in-flight groups (g-2 first so its AV runs
            # while g-1's softmax finishes)
            pm = softmax_head(pend_sm)
            av_tail(pend_av, pend_pmn)
            nxt_pmn = rowsum_part(pm)
            av_tail(pend_sm, nxt_pmn)
    nc.compile()
    return nc


def host_prep(gamma, beta, wq, bq, wk, bk, wv, bv, wp, bp):
    """Fold gamma/beta into weights; build all constant tensors."""
    s = 1.0 / np.sqrt(np.float64(C))
    g = gamma.astype(np.float64)

    def fold(w, bias, scale):
        a = (w.astype(np.float64) * g[None, :]) * scale      # (co, ci)
        u = (w.astype(np.float64) @ g) * scale               # (co,)
        c0 = (bias.astype(np.float64) + w.astype(np.float64) @
              beta.astype(np.float64)) * scale
        return a, u, c0

    aq, uq, cq = fold(wq, bq, s)
    ak, uk, ck = fold(wk, bk, 1.0)
    av, uv, cv = fold(wv, bv, 1.0)
    # scores are bilinear: S = (Ak x)^T (Aq x) = x^T G x with G = Ak^T Aq;
    # the surviving affine term (s-dependent only — t-terms cancel in
    # softmax) uses w1/w2: h = x^T Ak^T (cq - mu*r*uq)
    G = ak.T @ aq
    w1 = ak.T @ cq
    w2 = ak.T @ uq
    # output projection folded into V: Apv = Wp @ Av; P-eviction constant
    # dp = Wp@(cv - mu*r*uv) + bp = pv1 - mu*r*pv2
    wp64 = wp.astype(np.float64)
    apv = wp64 @ av
    pv1 = wp64 @ cv + bp.astype(np.float64)
    pv2 = wp64 @ uv
    gyt = np.ascontiguousarray(G.T).astype(BF)
    apvt = np.ascontiguousarray(apv.T).astype(BF)

    def colize(v):
        out = np.empty((P, CCH), np.float32)
        for ch in range(CCH):
            out[:, ch] = v[ch * P:(ch + 1) * P]
        return out

    w1c = colize(w1)
    w2c = colize(w2)
    pv1c = colize(pv1)
    pv2c = colize(pv2)

    # pair mask [128, 4*128]: diag 64x64 halves get causal triu (s<=t),
    # off-diag (cross-location) halves are zero; identical per pair.
    tri = np.triu(np.ones((T, T), np.float32))
    blkm = np.zeros((P, P), np.float32)
    blkm[0:T, 0:T] = tri
    blkm[T:2 * T, T:2 * T] = tri
    maskt = np.tile(blkm, (1, NPR))

    consts = {
        "wyt": gyt, "wpvt": apvt,
        "w1col": w1c, "w2col": w2c, "pv1col": pv1c, "pv2col": pv2c,
        "maskt": maskt.astype(BF),
        "ones_mat_b": np.ones((P, P), BF),
        "ones_col_b": np.ones((P, 1), BF),
    }
    return consts


_NC_CACHE = {}


def kernel(x, gamma, beta, wq, bq, wk, bk, wv, bv, wp, bp):
    x = np.asarray(x, np.float32)
    args = [np.asarray(a, np.float32) for a in
            (gamma, beta, wq, bq, wk, bk, wv, bv, wp, bp)]
    consts = host_prep(*args)

    if "nc" not in _NC_CACHE:
        _NC_CACHE["nc"] = build_nc()
    nc = _NC_CACHE["nc"]

    in_maps = []
    for core in range(NCORES):
        b, hg = core // 4, core % 4
        shard = x[b, :, :, hg * HSH:(hg + 1) * HSH, :]        # (C,T,HSH,W)
        shard = np.ascontiguousarray(
            shard.transpose(2, 0, 3, 1)).reshape(HSH, C, WT)  # w-major
        in_maps.append({"xs": shard.astype(BF), **consts})

    global _last_in_maps
    _last_in_maps = in_maps
    res = run_bass_kernel_spmd(nc, in_maps, list(range(NCORES)))

    out = np.empty((B, C, T, H, W), np.float32)
    for core in range(NCORES):
        b, hg = core // 4, core % 4
        o = res.results[core]["out"].reshape(HSH, C, W, T)
        out[b, :, :, hg * HSH:(hg + 1) * HSH, :] = o.transpose(1, 3, 0, 2)
    return out
